# revision 1
# baseline (speedup 1.0000x reference)
"""Sliding-window GQA attention with RoPE on 8 trn2 NeuronCores.

Sharding: core c = (b, g) with b = c // 4 (batch), g = c % 4 (kv-head group).
Each core computes its 4 query heads + 1 kv head for one batch element and
produces a partial output (its head-group's contribution to x @ Wo); the host
sums the 4 partials per batch.

All matmuls run in float32r (tf32-like, full PE rate at N>=256).
Layout strategy: activations kept transposed ([feature, seq]) so that
projections, scores (S^T), AV (U^T) and the output projection all consume
naturally-laid-out operands; softmax normalization is deferred to after AV.
"""

import math

import numpy as np

B, S, E = 2, 2048, 2048
H, KV, D = 16, 4, 128
WIN = 512
THETA = 10000.0
SCALE = 1.0 / math.sqrt(D)
NCORES = 8
GH = H // KV          # 4 query heads per kv group
MG = GH * D           # 512 q-features per group
ET = E // 128         # 16 e-tiles
ST = S // 128         # 16 s-tiles
NSC = S // 512        # 4 s-chunks / q-chunks

_CACHE = {}

# pool-buffer tuning knobs (consulted by _build_module)
_TUNE = {"xin": 2, "xT": 4, "rope": 4, "psT": 2, "att": 6, "psS": 4,
         "psU": 2, "psR": 2, "norm": 2, "osb": 4, "psO": 4, "psB": 2}

_BLOB_SPEC = [
    ("x", S * E), ("wq", E * MG), ("wk", E * D), ("wv", E * D),
    ("wo", MG * E), ("cos2", 128 * S), ("ssin", 128 * S),
    ("m1", 128 * 128), ("m2", 128 * 128), ("ones", 128),
    ("ident", 128 * 128), ("zeros", 128 * 512),
]
BLOB_OFF = {}
_off = 0
for _n, _ln in _BLOB_SPEC:
    BLOB_OFF[_n] = (_off, _ln)
    _off += (_ln + 2047) // 2048 * 2048
BLOB_TOT = _off


def _build_module(repA=1, repC=1, repD=1, dbg_no_rope=False, dbg_no_transpose=False, dbg_no_xdma=False):
    import concourse.bacc as bacc
    import concourse.bass as bass_mod
    import concourse.tile as tile
    import concourse.mybir as mybir

    F32R = mybir.dt.float32r
    F32 = mybir.dt.float32
    EXP = mybir.ActivationFunctionType.Exp

    nc = bacc.Bacc("TRN2", target_bir_lowering=False, debug=False,
                   enable_asserts=False, num_devices=NCORES)

    blob = nc.dram_tensor("blob", [BLOB_TOT], F32R, kind="ExternalInput").ap()

    def _view(name, shape, f32=False):
        off, ln = BLOB_OFF[name]
        v = blob[off:off + ln]
        if f32:
            v = v.bitcast(F32)
        import math as _m
        pat = " ".join(f"d{i}" for i in range(len(shape)))
        kw = {f"d{i}": shape[i] for i in range(len(shape))}
        return v.rearrange(f"({pat}) -> {pat}", **kw)

    x = _view("x", (S, E))
    wq = _view("wq", (E, MG))
    wk = _view("wk", (E, D))
    wv = _view("wv", (E, D))
    wo = _view("wo", (MG, E))
    cos2 = _view("cos2", (128, S), f32=True)
    ssin = _view("ssin", (128, S), f32=True)
    m1 = _view("m1", (128, 128))
    m2 = _view("m2", (128, 128))
    ones = _view("ones", (128, 1))
    ident = _view("ident", (128, 128))
    zeros = _view("zeros", (128, 512))
    out = nc.dram_tensor("out", [S, E], F32, kind="ExternalOutput").ap()
    rscr = nc.dram_tensor("rscr", [GH * NSC, 512], F32, kind="Internal").ap()

    with tile.TileContext(nc) as tc:
        with (
            tc.tile_pool(name="persist", bufs=1) as pp,
            tc.tile_pool(name="consts", bufs=1) as cp,
        ):
            # Persistent T-layout activations
            qt = pp.tile([128, GH, S], F32R, tag="qt")       # Q^T per head
            kt = pp.tile([128, S], F32R, tag="kt")           # K^T
            vt = pp.tile([128, S], F32R, tag="vt")           # V^T

            wq_sb = cp.tile([128, ET, MG], F32R, tag="wq")
            wk_sb = cp.tile([128, ET, D], F32R, tag="wk")
            wv_sb = cp.tile([128, ET, D], F32R, tag="wv")
            cos_sb = cp.tile([128, S], F32, tag="cos")
            sin_sb = cp.tile([128, S], F32, tag="sin")
            m1_sb = cp.tile([128, 128], F32R, tag="m1")
            m2_sb = cp.tile([128, 128], F32R, tag="m2")
            ones_sb = cp.tile([128, 1], F32R, tag="ones")
            id_sb = cp.tile([128, 128], F32R, tag="id")
            z_sb = cp.tile([128, 512], F32R, tag="z")

            nc.sync.dma_start(wq_sb[:], wq.rearrange("(t p) m -> p t m", p=128))
            nc.sync.dma_start(wk_sb[:], wk.rearrange("(t p) m -> p t m", p=128))
            nc.sync.dma_start(wv_sb[:], wv.rearrange("(t p) m -> p t m", p=128))
            nc.sync.dma_start(cos_sb[:], cos2[:])
            nc.sync.dma_start(sin_sb[:], ssin[:])
            nc.sync.dma_start(m1_sb[:], m1[:])
            nc.sync.dma_start(m2_sb[:], m2[:])
            nc.sync.dma_start(ones_sb[:], ones[:])
            nc.sync.dma_start(id_sb[:], ident[:])
            nc.sync.dma_start(z_sb[:], zeros[:])

            def phase_a():
                with (
                    tc.tile_pool(name="xin", bufs=_TUNE["xin"]) as xp,
                    tc.tile_pool(name="xT", bufs=_TUNE["xT"]) as xtp,
                    tc.tile_pool(name="rope", bufs=_TUNE["rope"]) as rp,
                    tc.tile_pool(name="psA", bufs=1, space="PSUM") as psa,
                    tc.tile_pool(name="psT", bufs=_TUNE["psT"], space="PSUM") as pst,
                ):
                    for sc in range(NSC):
                        s0 = sc * 512
                        xin = xp.tile([128, 4, E], F32R, tag="xin")
                        if not dbg_no_xdma:
                            for si in range(4):
                                nc.sync.dma_start(
                                    xin[:, si, :],
                                    x[s0 + si * 128: s0 + (si + 1) * 128, :])

                        q_ps = [psa.tile([128, 512], F32, name=f"qps{h}",
                                         tag=f"qps{h}") for h in range(GH)]
                        k_ps = psa.tile([128, 512], F32, tag="kps")
                        v_ps = psa.tile([128, 512], F32, tag="vps")

                        for et in range(ET):
                            xt_et = xtp.tile([128, 512], F32R, tag="xT")
                            if dbg_no_transpose:
                                nc.vector.tensor_copy(xt_et[:], xin[:, 0, 0:512])
                            else:
                                tp4 = pst.tile([128, 512], F32R, tag="tp")
                                for si in range(4):
                                    nc.tensor.transpose(
                                        tp4[:, si * 128:(si + 1) * 128],
                                        xin[:, si, et * 128:(et + 1) * 128], id_sb[:])
                                nc.vector.tensor_copy(xt_et[:], tp4[:])
                            first, last = et == 0, et == ET - 1
                            for h in range(GH):
                                nc.tensor.matmul(
                                    q_ps[h][:], wq_sb[:, et, h * 128:(h + 1) * 128],
                                    xt_et[:], start=first, stop=last)
                            nc.tensor.matmul(k_ps[:], wk_sb[:, et, :], xt_et[:],
                                             start=first, stop=last)
                            nc.tensor.matmul(v_ps[:], wv_sb[:, et, :], xt_et[:],
                                             start=first, stop=last)

                        # RoPE: rot(p) = p*cos + swap_halves(p)*sgn_sin
                        for h in range(GH + 1):
                            src = q_ps[h] if h < GH else k_ps
                            dst = qt[:, h, s0:s0 + 512] if h < GH else kt[:, s0:s0 + 512]
                            if dbg_no_rope:
                                nc.vector.tensor_copy(dst, src[:])
                                continue
                            a_t = rp.tile([128, 512], F32, tag="ropeA")
                            b_t = rp.tile([128, 512], F32, tag="ropeB")
                            nc.vector.tensor_mul(a_t[:], src[:], cos_sb[:, s0:s0 + 512])
                            nc.vector.tensor_mul(
                                b_t[0:64, :], src[64:128, :], sin_sb[0:64, s0:s0 + 512])
                            nc.vector.tensor_mul(
                                b_t[64:128, :], src[0:64, :], sin_sb[64:128, s0:s0 + 512])
                            nc.gpsimd.tensor_add(dst, a_t[:], b_t[:])
                        nc.vector.tensor_copy(vt[:, s0:s0 + 512], v_ps[:])

            for _ in range(repA):
                phase_a()

            # persist2: tensors whose lifetime starts after phase A
            pp2_cm = tc.tile_pool(name="persist2", bufs=1)
            pp2 = pp2_cm.__enter__()
            vn = pp2.tile([128, ST, 128], F32R, tag="vn")    # V natural, k-tiled
            ot = pp2.tile([128, GH, S], F32R, tag="ot")      # O^T per head
            wo_sb = pp2.tile([128, GH, E], F32R, tag="wo")
            nc.sync.dma_start(wo_sb[:], wo.rearrange("(f p) e -> p f e", p=128))

            # Phase B: V^T -> V natural (k on partitions)
            with tc.tile_pool(name="psB", bufs=_TUNE["psB"], space="PSUM") as psb:
                for t4 in range(ST // 4):
                    tp4 = psb.tile([128, 512], F32R, tag="tp")
                    for j in range(4):
                        t = t4 * 4 + j
                        nc.tensor.transpose(
                            tp4[:, j * 128:(j + 1) * 128],
                            vt[:, t * 128:(t + 1) * 128], id_sb[:])
                    nc.vector.tensor_copy(vn[:, t4 * 4:(t4 + 1) * 4, :], tp4[:])

            def phase_c():
                with (
                    tc.tile_pool(name="att", bufs=_TUNE["att"]) as ap_,
                    tc.tile_pool(name="norm", bufs=_TUNE["norm"]) as np_,
                    tc.tile_pool(name="psS", bufs=_TUNE["psS"], space="PSUM") as pss,
                    tc.tile_pool(name="psU", bufs=_TUNE["psU"], space="PSUM") as psu,
                    tc.tile_pool(name="psR", bufs=_TUNE["psR"], space="PSUM") as psr,
                ):
                    for h in range(GH):
                        for qc in range(NSC):
                            i0 = qc * 4
                            c0 = qc * 512
                            ts_ = list(range(max(0, i0 - 4), i0 + 4))
                            ut_ps = psu.tile([128, 512], F32, tag="ut")
                            r_ps = psr.tile([1, 512], F32, tag="r")
                            for idx, t in enumerate(ts_):
                                ilo, ihi = max(i0, t), min(i0 + 3, t + 4)
                                vlo, vhi = (ilo - i0) * 128, (ihi - i0 + 1) * 128
                                clo, chi = vlo, vhi
                                if chi - clo < 256:
                                    if clo >= 128:
                                        clo -= 128
                                    else:
                                        chi += 128
                                st_ps = pss.tile([128, 512], F32, tag="st")
                                nc.tensor.matmul(
                                    st_ps[:, clo:chi], kt[:, t * 128:(t + 1) * 128],
                                    qt[:, h, c0 + clo:c0 + chi], start=True, stop=True)
                                at = ap_.tile([128, 512], F32R, tag="at")
                                nc.scalar.activation(
                                    at[:, vlo:vhi], st_ps[:, vlo:vhi], EXP, scale=SCALE)
                                if t >= i0:  # causal (diagonal) mask at q-tile i = t
                                    bnd = (t - i0) * 128
                                    nc.vector.tensor_mul(
                                        at[:, bnd:bnd + 128], at[:, bnd:bnd + 128],
                                        m2_sb[:])
                                if i0 <= t + 4 <= i0 + 3:  # window-edge mask at i = t+4
                                    bnd = (t + 4 - i0) * 128
                                    nc.vector.tensor_mul(
                                        at[:, bnd:bnd + 128], at[:, bnd:bnd + 128],
                                        m1_sb[:])
                                if vlo > 0:
                                    nc.gpsimd.tensor_copy(at[:, 0:vlo], z_sb[:, 0:vlo])
                                if vhi < 512:
                                    nc.gpsimd.tensor_copy(at[:, vhi:512], z_sb[:, vhi:512])
                                first, last = idx == 0, idx == len(ts_) - 1
                                nc.tensor.matmul(ut_ps[:], vn[:, t, :], at[:],
                                                 start=first, stop=last)
                                nc.tensor.matmul(r_ps[:1, :], ones_sb[:], at[:],
                                                 start=first, stop=last)
                            rinv = np_.tile([1, 512], F32, tag="rinv")
                            nc.vector.reciprocal(rinv[:1, :], r_ps[:1, :])
                            hq = h * NSC + qc
                            nc.sync.dma_start(rscr[hq:hq + 1, :], rinv[:1, :])
                            rrep = np_.tile([128, 512], F32, tag="rrep")
                            row = rscr[hq:hq + 1, :]
                            bcast = bass_mod.AP(
                                tensor=row.tensor, offset=row.offset,
                                ap=[[0, 128]] + [list(p) for p in row.ap[1:]])
                            nc.sync.dma_start(rrep[:], bcast)
                            nc.vector.tensor_mul(ot[:, h, c0:c0 + 512], ut_ps[:], rrep[:])

            for _ in range(repC):
                phase_c()

            def phase_d():
                with (
                    tc.tile_pool(name="osb", bufs=_TUNE["osb"]) as op_,
                    tc.tile_pool(name="psO", bufs=_TUNE["psO"], space="PSUM") as pso,
                ):
                    for st_i in range(ST):
                        r0 = st_i * 128
                        for eo in range(4):
                            e0 = eo * 512
                            o_ps = pso.tile([128, 512], F32, tag="ops")
                            for f in range(GH):
                                nc.tensor.matmul(
                                    o_ps[:], ot[:, f, r0:r0 + 128],
                                    wo_sb[:, f, e0:e0 + 512],
                                    start=(f == 0), stop=(f == GH - 1))
                            o_sb = op_.tile([128, 512], F32, tag="osb")
                            nc.vector.tensor_copy(o_sb[:], o_ps[:])
                            nc.sync.dma_start(out[r0:r0 + 128, e0:e0 + 512], o_sb[:])

            for _ in range(repD):
                phase_d()

            pp2_cm.__exit__(None, None, None)

    nc.compile()
    return nc


def _host_constants():
    pos = np.arange(S, dtype=np.float64)
    inv = 1.0 / (THETA ** (np.arange(0, D, 2, dtype=np.float64) / D))  # [64]
    ang = inv[:, None] * pos[None, :]                                   # [64, S]
    cos2 = np.concatenate([np.cos(ang), np.cos(ang)], 0).astype(np.float32)
    ssin = np.concatenate([-np.sin(ang), np.sin(ang)], 0).astype(np.float32)
    jj = np.arange(128)[:, None]
    qq = np.arange(128)[None, :]
    m1 = (jj >= qq + 1).astype(np.float32)
    m2 = (jj <= qq).astype(np.float32)
    ones = np.ones((128, 1), np.float32)
    ident = np.eye(128, dtype=np.float32)
    zeros = np.zeros((128, 512), np.float32)
    return cos2, ssin, m1, m2, ones, ident, zeros


def _get_runner():
    if "run" in _CACHE:
        return _CACHE["run"]
    import jax
    import jax.numpy as jnp
    from jax.sharding import Mesh, PartitionSpec
    from jax.experimental.shard_map import shard_map
    import concourse.mybir as mybir_m
    from concourse.bass2jax import _bass_exec_p, install_neuronx_cc_hook, partition_id_tensor

    install_neuronx_cc_hook()
    nc = _build_module()

    partition_name = nc.partition_id_tensor.name if nc.partition_id_tensor else None
    in_names, out_names, out_avals, out_shapes = [], [], [], []
    for alloc in nc.m.functions[0].allocations:
        if not isinstance(alloc, mybir_m.MemoryLocationSet):
            continue
        name = alloc.memorylocations[0].name
        if alloc.kind == "ExternalInput":
            if name != partition_name:
                in_names.append(name)
        elif alloc.kind == "ExternalOutput":
            out_names.append(name)
            shape = tuple(alloc.tensor_shape)
            dtype = mybir_m.dt.np(alloc.dtype)
            out_avals.append(jax.core.ShapedArray(shape, dtype))
            out_shapes.append((shape, dtype))
    assert in_names == ["blob"] and out_names == ["out"], (in_names, out_names)
    n_params = 1
    all_names = list(in_names) + out_names
    if partition_name is not None:
        all_names.append(partition_name)

    def _body(*args):
        operands = list(args)
        if partition_name is not None:
            operands.append(partition_id_tensor())
        outs = _bass_exec_p.bind(
            *operands,
            out_avals=tuple(out_avals),
            in_names=tuple(all_names),
            out_names=tuple(out_names),
            lowering_input_output_aliases=(),
            sim_require_finite=False,
            sim_require_nnan=False,
            nc=nc,
        )
        return tuple(outs)

    devices = jax.devices()[:NCORES]
    mesh = Mesh(np.asarray(devices), ("core",))
    sharded = shard_map(_body, mesh=mesh,
                        in_specs=(PartitionSpec("core"),) * 2,
                        out_specs=(PartitionSpec("core"),), check_rep=False)

    jf = jax.jit(sharded, donate_argnums=(1,), keep_unused=True)
    jred = jax.jit(lambda o: o.reshape(B, KV, S, E).sum(axis=1))
    mkzeros = jax.jit(lambda: jnp.zeros((NCORES * S, E), np.float32))

    from jax.sharding import NamedSharding

    def _pad_flat(a, ln_pad):
        f = a.reshape(-1)
        pad = ln_pad - f.shape[0]
        return jnp.pad(f, (0, pad)) if pad else f

    _pl = {n: (BLOB_OFF[n][0], (_ln + 2047) // 2048 * 2048)
           for n, _ln in _BLOB_SPEC}

    def _asm8(x_full, Wq, Wk, Wv, Wo, consts):
        cos2, ssin, m1, m2, ones_, ident, zeros_ = consts
        shards = []
        for c in range(NCORES):
            b, g = c // KV, c % KV
            pieces = [
                _pad_flat(x_full[b], _pl["x"][1]),
                _pad_flat(Wq[:, g * MG:(g + 1) * MG], _pl["wq"][1]),
                _pad_flat(Wk[:, g * D:(g + 1) * D], _pl["wk"][1]),
                _pad_flat(Wv[:, g * D:(g + 1) * D], _pl["wv"][1]),
                _pad_flat(Wo[g * MG:(g + 1) * MG, :], _pl["wo"][1]),
                _pad_flat(cos2, _pl["cos2"][1]),
                _pad_flat(ssin, _pl["ssin"][1]),
                _pad_flat(m1, _pl["m1"][1]),
                _pad_flat(m2, _pl["m2"][1]),
                _pad_flat(ones_, _pl["ones"][1]),
                _pad_flat(ident, _pl["ident"][1]),
                _pad_flat(zeros_, _pl["zeros"][1]),
            ]
            shards.append(jnp.concatenate(pieces))
        return shards

    asm8 = jax.jit(_asm8)
    blob_sharding = NamedSharding(mesh, PartitionSpec("core"))

    def put_inputs(raw):
        """raw = (x_full, Wq, Wk, Wv, Wo) as float32 numpy arrays."""
        import hashlib
        h = hashlib.blake2b(digest_size=16)
        for a in raw:
            h.update(np.ascontiguousarray(a, np.float32).tobytes())
            h.update(str(a.shape).encode())
        key = h.hexdigest()
        if _CACHE.get("blob_key") == key:
            return [_CACHE["blob_dev"]]
        dev0 = devices[0]
        placed = [jax.device_put(np.ascontiguousarray(a, np.float32), dev0)
                  for a in raw]
        if "consts_dev0" not in _CACHE:
            consts = _CACHE.setdefault("consts", _host_constants())
            _CACHE["consts_dev0"] = [jax.device_put(c, dev0) for c in consts]
        shards = asm8(*placed, _CACHE["consts_dev0"])
        shard_arrs = [shards[0]] + [
            jax.device_put(shards[c], devices[c]) for c in range(1, NCORES)]
        for a in shard_arrs:
            a.block_until_ready()
        blob = jax.make_array_from_single_device_arrays(
            (NCORES * BLOB_TOT,), blob_sharding, shard_arrs)
        _CACHE["blob_key"] = key
        _CACHE["blob_dev"] = blob
        return [blob]

    def execute(ins_dev):
        zeros = mkzeros()
        (o,) = jf(ins_dev[0], zeros)
        r = jred(o)
        jax.block_until_ready(r)
        return r

    def fetch(o):
        return np.asarray(o)

    def runner(raw):
        return fetch(execute(put_inputs(raw)))

    runner.put_inputs = put_inputs
    runner.execute = execute
    runner.fetch = fetch
    _CACHE["run"] = runner
    return runner


def _make_in_maps(x_full, Wq, Wk, Wv, Wo):
    consts = _CACHE.setdefault("consts", _host_constants())
    cos2, ssin, m1, m2, ones, ident, zeros = consts
    in_maps = []
    for c in range(NCORES):
        b, g = c // KV, c % KV
        in_maps.append({
            "x": np.ascontiguousarray(x_full[b], np.float32),
            "wq": np.ascontiguousarray(Wq[:, g * MG:(g + 1) * MG], np.float32),
            "wk": np.ascontiguousarray(Wk[:, g * D:(g + 1) * D], np.float32),
            "wv": np.ascontiguousarray(Wv[:, g * D:(g + 1) * D], np.float32),
            "wo": np.ascontiguousarray(Wo[g * MG:(g + 1) * MG, :], np.float32),
            "cos2": cos2, "ssin": ssin, "m1": m1, "m2": m2,
            "ones": ones, "ident": ident, "zeros": zeros,
        })
    return in_maps


def kernel(x, Wq, Wk, Wv, Wo):
    run = _get_runner()
    raw = tuple(np.asarray(a, np.float32) for a in (x, Wq, Wk, Wv, Wo))
    out = run(raw)
    return np.ascontiguousarray(out, dtype=np.float32)

